# revision 1
# baseline (speedup 1.0000x reference)
"""Trainium2 Bass kernel for the dual-stream "DifAttention" block — v3.

Partitioning (unchanged from baseline): 8 independent (batch, stream) units,
one per core, SPMD, no collectives:
    x-core b: t_qk=x[b], t_v=x[b], t_qo=y[b]
    y-core b: t_qk=y[b], t_v=x[b], t_qo=x[b]

PE-cycle reduction, accuracy-constrained (plain fp8 anywhere on the softmax
path measures ~2e-2 absmax error alone — over the gate — so every fp8 use
carries a residual correction):

  All projections   split-fp8 DoubleRow: q = x8@w8 + x8@dw8 + dx8@w8 where
                    x8=fp8(x), dx8=fp8(x-x8) (host-prepped, e4m3 subnormals
                    carry the residual), w8=fp8(32 w), dw8=fp8(32w - w8).
                    3 DR terms x 3 k-pairs = 27648 cyc/proj vs 36864 bf16,
                    with ~bf16 accuracy (residual ~0.1%).
  S^T = K Q^T       fp8 DR, one instr per (head, m-tile, chunk): stationary
                    [128k x 2i x 128M]: i=0 blockdiag(k8[mA]|k8[mB]), i=1
                    the SAME blockdiag of dk8 = fp32K - k8 (the otherwise
                    wasted pair slot corrects K to full precision); moving =
                    [q8;q8] partition-duplicated, 0-stride i-repeat.
                    8192 cyc/unit = half of bf16; only q8's cast error
                    (~1e-2 worst-case absmax) survives.
  A = exp(S/8192)   ACT -> bf16 (the pipeline pacer, ~1.15us/tile)
  O = A V           bf16, o[n,d] formulation: stationary = A^T tile
                    [128m x 128n] (FWL loads it in ~32 cyc, hidden), moving
                    = V[m, 64d + ones-col] (65 cols) -> full 128-partition
                    output at 65 cyc/matmul = half the o^T-form cost, and
                    the softmax denominator lands as a per-PARTITION column
                    so normalize needs no DMA-bounce broadcast.
  out proj          bf16 from onorm^T (PE-transposed via identity matmuls).
"""

import numpy as np
import ml_dtypes

import concourse.bass as bass
import concourse.bacc as bacc
import concourse.tile as tile
from concourse import mybir
from concourse.bass_utils import run_bass_kernel_spmd

P = 128
B, N, C = 4, 1024, 768
H, HD = 12, 64
CT = C // P           # 6 column tiles (= head pairs)
NT = N // P           # 8 sequence tiles
WSCALE = 32.0         # host pre-scale on Wq/Wk/Wqo (fp8 subnormal avoidance)
EXPSC = 0.125 / (WSCALE * WSCALE)   # = 1/8192, folds the x32*x32 back out

FP32 = mybir.dt.float32
BF16 = mybir.dt.bfloat16
FP8 = mybir.dt.float8e4
DR = mybir.MatmulPerfMode.DoubleRow
EXP = mybir.ActivationFunctionType.Exp
SUB = mybir.AluOpType.subtract


def _rep2(sl):
    """Inject a 0-stride pair dim into a [128, n] AP -> [128, 2, n]."""
    return bass.AP(tensor=sl.tensor, offset=sl.offset,
                   ap=[list(sl.ap[0]), [0, 2], list(sl.ap[1])])


def build_kernel():
    nc = bacc.Bacc("TRN2", target_bir_lowering=False, debug=False,
                   num_devices=8)

    d_qk = nc.dram_tensor("qk8", [C, 2, N], FP8, kind="ExternalInput")
    d_qo = nc.dram_tensor("qo8", [C, 2, N], FP8, kind="ExternalInput")
    d_v = nc.dram_tensor("v8", [C, 2, N], FP8, kind="ExternalInput")
    d_wq = nc.dram_tensor("wq8", [C, 2, C], FP8, kind="ExternalInput")
    d_wk = nc.dram_tensor("wk8", [C, 2, C], FP8, kind="ExternalInput")
    d_wqo = nc.dram_tensor("wqo8", [C, 2, C], FP8, kind="ExternalInput")
    d_wv = nc.dram_tensor("wv8", [C, 2, C], FP8, kind="ExternalInput")
    d_wp = nc.dram_tensor("wp16", [C, C], BF16, kind="ExternalInput")
    d_eye = nc.dram_tensor("eye16", [P, P], BF16, kind="ExternalInput")
    d_zero = nc.dram_tensor("zeros8", [1, 12288], FP8, kind="ExternalInput")
    d_out = nc.dram_tensor("out", [N, C], FP32, kind="ExternalOutput")

    with tile.TileContext(nc) as tc:
        _body(tc, d_qk, d_qo, d_v, d_wq, d_wk, d_wqo, d_wv, d_wp, d_eye,
              d_zero, d_out)
    nc.compile()
    return nc


def _body(tc, d_qk, d_qo, d_v, d_wq, d_wk, d_wqo, d_wv, d_wp, d_eye,
          d_zero, d_out):
    nc = tc.nc
    _ap = lambda t: t if isinstance(t, bass.AP) else t.ap()
    d_qk, d_qo, d_v, d_wq, d_wk, d_wqo, d_wv, d_wp, d_eye, d_zero, d_out = (
        _ap(t) for t in (d_qk, d_qo, d_v, d_wq, d_wk, d_wqo, d_wv, d_wp,
                         d_eye, d_zero, d_out))
    from contextlib import ExitStack
    ctx = ExitStack()
    xpool = ctx.enter_context(tc.tile_pool(name="xpool", bufs=2))
    wpool = ctx.enter_context(tc.tile_pool(name="wpool", bufs=2))
    spool = ctx.enter_context(tc.tile_pool(name="spool", bufs=3))
    persist = ctx.enter_context(tc.tile_pool(name="persist", bufs=1))
    apool = ctx.enter_context(tc.tile_pool(name="apool", bufs=2))
    rpool = ctx.enter_context(tc.tile_pool(name="rpool", bufs=2))
    tpool = ctx.enter_context(tc.tile_pool(name="tpool", bufs=2))
    opool = ctx.enter_context(tc.tile_pool(name="opool", bufs=2))
    psA = ctx.enter_context(tc.tile_pool(name="psA", bufs=4, space="PSUM"))

    # ---- persistent tensors -------------------------------------------------
    # kblk[k, co, h, mt, i, M]: the [128,2,128] DR stationary per (co,h,mt):
    #   i=0: blockdiag(k8[d,mA] rows 0-63 x cols 0-63, k8[d,mB] rows 64-127
    #        x cols 64-127);  i=1: same blockdiag of dk8 (K residual)
    kblk = persist.tile([P, CT, 2, NT, 2, P], FP8, name="kblk")
    qdup = persist.tile([P, CT, 2, N], FP8, name="qdup")
    qodup = persist.tile([P, CT, 2, N], FP8, name="qodup")
    # V[m, head, d] bf16 with a ones column at d=64 (softmax denominator)
    vsb = persist.tile([P, NT, H, HD + 1], BF16, name="vsb")
    onorm = persist.tile([P, NT, C], BF16, name="onorm")   # O[n, c]
    onormT = persist.tile([P, CT, N], BF16, name="onormT")  # O^T[c, n]
    eye = persist.tile([P, P], BF16, name="eye")

    # zero the off-diagonal blocks of kblk (both i slots) on DVE, one
    # granule per co so the first S matmul (co=0) unblocks early
    def zmemset(co, parts, col_off):
        base = kblk[parts, co, 0, 0, 0, col_off:col_off + 64]
        dst = bass.AP(tensor=base.tensor, offset=base.offset,
                      ap=[list(base.ap[0]), [128, 32], [1, 64]])
        nc.vector.memset(dst, 0.0)

    def zmemset_co(co):
        zmemset(co, slice(0, 64), 64)
        zmemset(co, slice(64, P), 0)

    # ---- load inputs --------------------------------------------------------
    # x tensors arrive as [C, 2, N] fp8: slot 0 = fp8(x^T), slot 1 = residual
    xqk = xpool.tile([P, CT, 2, N], FP8, tag="x8", name="xqk")
    xqo = xpool.tile([P, CT, 2, N], FP8, tag="x8", name="xqo")
    xv = xpool.tile([P, CT, 2, N], FP8, tag="x8b", name="xv", bufs=1)
    for xt, dt in ((xqk, d_qk), (xv, d_v), (xqo, d_qo)):
        nc.sync.dma_start(xt[:], dt.rearrange("(t p) v n -> p t v n", p=P))
    nc.sync.dma_start(eye[:], d_eye)

    # ---- phase 1: projections (split-fp8 DoubleRow), co-granular ----------
    def make_proj(d_w, srcx, sink, name, eng=None, lite=False):
        wsb = wpool.tile([P, CT, 2, C], FP8, tag="w8", name=name, bufs=3)
        (eng or nc.gpsimd).dma_start(
            wsb[:], d_w.rearrange("(t p) v co -> p t v co", p=P))
        # lite: skip the residual terms. Used for QO, whose output is
        # quantized to fp8 for the S matmul anyway (unlike K, which gets the
        # dk8 slot correction), so the residuals add nothing the cast keeps.
        terms = [(0, 0), (1, 0)] if lite else [(0, 0), (1, 0), (0, 1)]
        tlast = len(terms) - 1

        def co_fn(co):
            ps = psA.tile([P, N], FP32, tag="s", name="ps_qkv")
            cosl = slice(co * P, (co + 1) * P)
            for ch in range(2):
                nsl = slice(ch * 512, (ch + 1) * 512)
                for ti, (wv_, xv_) in enumerate(terms):
                    for j in range(3):
                        nc.tensor.matmul(
                            ps[:, nsl],
                            wsb[:, 2 * j:2 * j + 2, wv_, cosl],
                            srcx[:, 2 * j:2 * j + 2, xv_, nsl],
                            start=(ti == 0 and j == 0),
                            stop=(ti == tlast and j == 2), perf_mode=DR)
            sink(co, ps)
        return co_fn

    def dup_sink(dup):
        def sink(co, ps):
            stg = spool.tile([P, N], FP8, tag="stg", name="qstg")
            nc.vector.tensor_copy(stg[:], ps[:])
            nc.gpsimd.dma_start(dup[0:64, co, 0, :], stg[0:64, :])
            nc.gpsimd.dma_start(dup[64:P, co, 0, :], stg[0:64, :])
            nc.gpsimd.dma_start(dup[0:64, co, 1, :], stg[64:P, :])
            nc.gpsimd.dma_start(dup[64:P, co, 1, :], stg[64:P, :])
        return sink

    def k_sink(co, ps):
        k8 = spool.tile([P, N], FP8, tag="stg", name="k8stg")
        dk8 = spool.tile([P, N], FP8, tag="stg", name="dk8stg")

        def stg_ap(stg, prt, half):
            s = stg[prt, half * 64:half * 64 + 64]
            return bass.AP(tensor=s.tensor, offset=s.offset,
                           ap=[list(s.ap[0]), [128, NT], [1, 64]])

        def scatter(i, stg):
            nc.gpsimd.dma_start(kblk[0:64, co, 0, :, i, 0:64],
                                stg_ap(stg, slice(0, 64), 0))
            nc.gpsimd.dma_start(kblk[64:P, co, 0, :, i, 64:P],
                                stg_ap(stg, slice(0, 64), 1))
            nc.gpsimd.dma_start(kblk[0:64, co, 1, :, i, 0:64],
                                stg_ap(stg, slice(64, P), 0))
            nc.gpsimd.dma_start(kblk[64:P, co, 1, :, i, 64:P],
                                stg_ap(stg, slice(64, P), 1))
        nc.vector.tensor_copy(k8[:], ps[:])
        scatter(0, k8)
        nc.vector.tensor_tensor(dk8[:], ps[:], k8[:], SUB)
        scatter(1, dk8)

    def make_vproj():
        wsb = wpool.tile([P, CT, 2, C], FP8, tag="w8", name="wv8", bufs=3)
        nc.gpsimd.dma_start(wsb[:],
                            d_wv.rearrange("(t p) v co -> p t v co", p=P))

        def mt_fn(mt):
            ps = psA.tile([P, N], FP32, tag="s", name="ps_v")
            msl = slice(mt * P, (mt + 1) * P)
            for base, wd in ((0, 512), (512, 256)):
                terms = [(0, 0), (0, 1), (1, 0)]  # (x-slot, w-slot)
                for ti, (xv_, wv_) in enumerate(terms):
                    for j in range(3):
                        nc.tensor.matmul(
                            ps[:, base:base + wd],
                            xv[:, 2 * j:2 * j + 2, xv_, msl],
                            wsb[:, 2 * j:2 * j + 2, wv_, base:base + wd],
                            start=(ti == 0 and j == 0),
                            stop=(ti == 2 and j == 2), perf_mode=DR)
            nc.vector.tensor_scalar_mul(
                vsb[:, mt, :, 0:HD],
                ps[:, 0:C].rearrange("p (h d) -> p h d", h=H),
                1.0 / WSCALE)
        return mt_fn

    # ---- phase 2: attention, pipelined at (p, att, head) granularity --------
    # The AV matmuls of the previous head are emitted as 8 per-nt slices
    # woven between this head's S/exp pairs, so the in-order PE queue always
    # has ready work while it waits for exp to free s-psum tiles.
    def emit_av_slice(pend, nt):
        pp, patt, ph, a, o = pend
        hh = 2 * pp + ph
        ntsl = slice(nt * P, (nt + 1) * P)
        for mt in range(NT):
            nc.tensor.matmul(
                o[:, nt, 0:HD + 1], a[:, mt, ntsl],
                vsb[:, mt, hh, :],
                start=(mt == 0), stop=(mt == NT - 1),
                skip_group_check=True)

    def emit_norm(pend):
        pp, patt, ph, a, o = pend
        hh = 2 * pp + ph
        # normalize: denominator is column HD -> per-partition scalar
        r = rpool.tile([P, NT, 1], FP32, tag="r", name="r_den")
        nc.vector.reciprocal(r[:], o[:, :, HD:HD + 1])
        rb = bass.AP(tensor=r.tensor, offset=r[:].offset,
                     ap=[list(r[:].ap[0]), [1, NT], [0, HD]])
        dst = onorm[:, :, hh * HD:(hh + 1) * HD]
        if patt == 0:
            nc.vector.tensor_mul(dst, o[:, :, 0:HD], rb)
        else:
            t = tpool.tile([P, NT, HD], BF16, tag="t", name="t_norm")
            nc.vector.tensor_mul(t[:], o[:, :, 0:HD], rb)
            nc.vector.tensor_add(dst, dst, t[:])

    def emit_transpose(p):
        trp = psA.tile([P, N], BF16, tag="s", name="tr")
        for nt in range(NT):
            nc.tensor.transpose(trp[:, nt * P:(nt + 1) * P],
                                onorm[:, nt, p * P:(p + 1) * P], eye[:])
        nc.vector.tensor_copy(onormT[:, p, :], trp[:])

    fillers = []

    def pump():
        if fillers:
            fillers.pop(0)()

    def emit_head(p, att, h, pend, pumps):
        qsrc = qdup if att == 0 else qodup
        sgn = EXPSC if att == 0 else -EXPSC
        a = apool.tile([P, NT, N], BF16, tag="a", name="a_att")
        o = None
        if pend is not None:
            o = psA.tile([P, NT, P], FP32, tag="s", name="o_av")
            pend = pend + (o,)
        for mt in range(NT):
            s = psA.tile([P, N], FP32, tag="s", name="s_att")
            for ch in range(2):
                nsl = slice(ch * 512, (ch + 1) * 512)
                nc.tensor.matmul(
                    s[:, nsl], kblk[:, p, h, mt, :, :],
                    _rep2(qsrc[:, p, h, nsl]),
                    start=True, stop=True, perf_mode=DR)
            nc.scalar.activation(a[:, mt, :], s[:], EXP, scale=sgn)
            if pend is not None:
                emit_av_slice(pend, mt)
                if mt == NT - 1:
                    emit_norm(pend)
                    if pend[1] == 1 and pend[2] == 1:
                        emit_transpose(pend[0])
            for _ in range(pumps(mt)):
                pump()
        return a

    # weight loads: wq/wk on the scalar/vector DMA queues (idle at t=0) so
    # they land in parallel with the x loads on sync and the rest on gpsimd
    kco = make_proj(d_wk, xqk, k_sink, "wk8")
    qco = make_proj(d_wq, xqk, dup_sink(qdup), "wq8", eng=nc.scalar)
    vmt = make_vproj()
    qoco = make_proj(d_wqo, xqo, dup_sink(qodup), "wqo8", lite=True)
    wp = wpool.tile([P, CT, C], BF16, tag="wf", name="wp", bufs=1)
    nc.gpsimd.dma_start(wp[:], d_wp.rearrange("(t p) co -> p t co", p=P))

    # co=0 of Q and K inline, the rest trickles through the filler pump so
    # the exp stream starts as early as possible and the PE never starves it.
    # Order matters doubly: emission order IS dependency order for readers
    # emitted later, and each (p,*) head needs its co=p slices beforehand.
    kco(0)
    qco(0)
    # DVE zero-fills AFTER the co0 casts so they don't block the kblk chain
    nc.vector.memset(vsb[:, :, :, HD:HD + 1], 1.0)
    zmemset_co(0)
    # pre-head V tiles: absorbed while the co0 cast/scatter chain completes
    for mt in range(4):
        vmt(mt)
    for mt in range(4, NT):
        fillers.append(lambda mt=mt: vmt(mt))
    fillers.append(lambda: qco(4))
    fillers.append(lambda: kco(4))
    fillers.append(lambda: zmemset_co(4))
    fillers.append(lambda: qco(1))
    fillers.append(lambda: kco(1))
    fillers.append(lambda: zmemset_co(1))
    fillers.append(lambda: qco(2))
    fillers.append(lambda: kco(2))
    fillers.append(lambda: zmemset_co(2))
    fillers.append(lambda: qco(3))
    fillers.append(lambda: kco(3))
    fillers.append(lambda: zmemset_co(3))
    for c in range(4):
        fillers.append(lambda c=c: qoco(c))
    fillers.append(lambda: qoco(4))
    fillers.append(lambda: qoco(5))
    fillers.append(lambda: qco(5))
    fillers.append(lambda: kco(5))
    fillers.append(lambda: zmemset_co(5))

    heads = [(0, 0, 0), (0, 0, 1), (4, 0, 0), (4, 0, 1), (1, 0, 0),
             (1, 0, 1), (0, 1, 0), (0, 1, 1), (4, 1, 0), (4, 1, 1),
             (2, 0, 0), (2, 0, 1), (1, 1, 0), (1, 1, 1), (3, 0, 0),
             (3, 0, 1), (2, 1, 0), (2, 1, 1), (5, 0, 0), (5, 0, 1),
             (3, 1, 0), (3, 1, 1), (5, 1, 0), (5, 1, 1)]
    pend = None
    for idx, (p, att, h) in enumerate(heads):
        # unit 0 has no AV slices -> drain one filler per mt; afterwards one
        # filler every other mt keeps the PE just behind the exp pace
        pumps = (lambda mt: 1) if idx == 0 else (lambda mt: mt % 2)
        a = emit_head(p, att, h, pend, pumps)
        pend = (p, att, h, a)
    o = psA.tile([P, NT, P], FP32, tag="s", name="o_av")
    pend = pend + (o,)
    for nt in range(NT):
        emit_av_slice(pend, nt)
    emit_norm(pend)
    emit_transpose(pend[0])

    # ---- phase 3: output projection -----------------------------------------
    def proj_partial(ps, nt, cts, start, stop):
        for base, wd in ((0, 512), (512, 256)):
            for ct in cts:
                nc.tensor.matmul(
                    ps[:, base:base + wd],
                    onormT[:, ct, nt * P:(nt + 1) * P],
                    wp[:, ct, base:base + wd],
                    start=(start and ct == cts[0]),
                    stop=(stop and ct == cts[-1]))

    def proj_store(ps, nt):
        osb = opool.tile([P, C], FP32, tag="out", name="osb")
        nc.vector.tensor_copy(osb[:], ps[:, 0:C])
        nc.sync.dma_start(d_out[nt * P:(nt + 1) * P, :], osb[:])

    pss = [psA.tile([P, N], FP32, tag="s", name="ps_proj%d" % i)
           for i in range(4)]
    for i in range(4):
        proj_partial(pss[i], i, list(range(CT - 1)), start=True, stop=False)
    for i in range(4):
        proj_partial(pss[i], i, [CT - 1], start=False, stop=True)
        proj_store(pss[i], i)
    for nt in range(4, NT):
        ps = psA.tile([P, N], FP32, tag="s", name="ps_proj")
        proj_partial(ps, nt, list(range(CT)), start=True, stop=True)
        proj_store(ps, nt)

    ctx.close()


_NC = None


def _get_nc():
    global _NC
    if _NC is None:
        _NC = build_kernel()
    return _NC


def _split8(a):
    """[R, Cc] fp32 -> [R, 2, Cc] fp8: (fp8(a), fp8(a - fp8(a)))."""
    f8 = ml_dtypes.float8_e4m3
    a8 = a.astype(f8)
    da = (a - a8.astype(np.float32)).astype(f8)
    return np.ascontiguousarray(np.stack([a8, da], axis=1))


def prepare_in_maps(x, y, w_qkv, w_proj, b_proj):
    x = np.asarray(x, np.float32)
    y = np.asarray(y, np.float32)
    w_qkv = np.asarray(w_qkv, np.float32)
    w_proj = np.asarray(w_proj, np.float32)

    bf = ml_dtypes.bfloat16
    spw = lambda w: _split8(np.ascontiguousarray(w.T) * WSCALE)
    spx = lambda t: _split8(np.ascontiguousarray(t.T))
    wqo8 = spw(w_qkv[0:C])
    wq8 = spw(w_qkv[C:2 * C])
    wk8 = spw(w_qkv[2 * C:3 * C])
    wv8 = spw(w_qkv[3 * C:4 * C])
    wp16 = np.ascontiguousarray(w_proj.T).astype(bf)
    eye16 = np.eye(P, dtype=bf)
    zeros8 = np.zeros((1, 12288), dtype=ml_dtypes.float8_e4m3)

    in_maps = []
    for i in range(8):
        b = i % 4
        isx = i < 4
        t_qk = x[b] if isx else y[b]
        t_qo = y[b] if isx else x[b]
        in_maps.append({
            "qk8": spx(t_qk), "qo8": spx(t_qo), "v8": spx(x[b]),
            "wq8": wq8, "wk8": wk8, "wqo8": wqo8, "wv8": wv8,
            "wp16": wp16, "eye16": eye16, "zeros8": zeros8,
        })
    return in_maps


def kernel(x, y, w_qkv, w_proj, b_proj):
    nc = _get_nc()
    in_maps = prepare_in_maps(x, y, w_qkv, w_proj, b_proj)
    res = run_bass_kernel_spmd(nc, in_maps, list(range(8)))
    bpf = np.asarray(b_proj, np.float32)
    out_x = np.stack([res.results[b]["out"] for b in range(4)]) + bpf
    out_y = np.stack([res.results[4 + b]["out"] for b in range(4)]) + bpf
    return out_x.astype(np.float32), out_y.astype(np.float32)


if __name__ == "__main__":
    rng = np.random.default_rng(0)
    ins = {
        "x": rng.standard_normal((B, N, C), dtype=np.float32),
        "y": rng.standard_normal((B, N, C), dtype=np.float32),
        "w_qkv": (rng.standard_normal((4 * C, C)) * 0.02).astype(np.float32),
        "w_proj": (rng.standard_normal((C, C)) * 0.02).astype(np.float32),
        "b_proj": (rng.standard_normal(C) * 0.02).astype(np.float32),
    }
    ox, oy = kernel(**ins)
    print(ox.shape, oy.shape, ox.dtype)

